# revision 23
# baseline (speedup 1.0000x reference)
"""Paged GQA decode attention (sparse_attention) on 8 TRN2 NeuronCores.

Slot-parallel streaming design: the page table is (in the graded problem) a
permutation of all 32768 cache slots, so the full K/V caches are read exactly
once.  Core i owns the contiguous slot range [i*4096, (i+1)*4096) and STREAMS
it sequentially at the ~358 GB/s per-core HBM roofline on the SP HWDGE queue,
in chunks of r*128 slots ([128 partitions, r rows, 1024]); r<=2 keeps chunk
arrival granularity at or below the per-group compute cadence so the PE never
sits idle waiting for a large chunk to land.

Each core scores its slots against ALL 16 sequences' queries, multiplies by a
host-built per-(slot, seq) ownership weight (0/1 under the permutation;
counts for general inputs), and accumulates the partial attention numerator
and denominator in PSUM over the whole stream.  The 16 new-token k/v rows are
host-transposed (knewT) and processed as a cheap 16-partition group with NO
on-device transposes, emitted mid-pipeline (after streamed group 2) so the
tensor engine never stalls waiting for its constants; its ownership weights
arrive as ln-weights accumulated into the score PSUM through a one-hot
[16,j]^T @ [16,qcols] matmul so Exp alone masks foreign rows.  The streamed
ownership weight of a replaced slot is zeroed on the core that owns it.  No
gathers, no scatters -- one fully static program per core.  The host sums the
per-core partials (flash-decoding split-K combine) and normalizes.

Per-instruction HW profiling showed the tensor engine is cadence-bound at
roughly max(ldweights, matmul-stream) ~ 1ns/column, and Vector/Scalar
instructions cost ~690ns nearly independent of size, so the pipeline
minimizes PE columns moved and DVE/ACT instruction count:
  - everything on the PE is bf16: K tiles are converted f32->bf16 via the
    transpose PSUM->SBUF copies (one on Vector, one on Scalar) and V tiles
    by one Scalar + one Vector copy, two groups ahead of their consumers,
  - PV is flipped (lhsT = masked-P, 64-column weight loads; V is the moving
    operand) and V carries an appended ones-column per head so each PV
    matmul also emits that head's denominator -- no separate den matmul,
  - PSUM accumulators are memset once and PV runs start=stop=False
    (skip_group_check): TRN2 zeroes lazily per 2KB region on
    start_tensor_calc, so 8 per-head groups in one bank would clobber each
    other.
The output numerator is copied and DMA'd out in two halves so the second
half's transfer latency is all that remains after the last PV.
"""

import ml_dtypes
import numpy as np

# ---- problem constants (must match the harness's reference.py) ----
NUM_HEADS = 32
NUM_KV_HEADS = 8
HEAD_DIM = 128
BS = 16
KV_LEN = 2048
NUM_SLOTS = BS * KV_LEN          # 32768
D = NUM_KV_HEADS * HEAD_DIM      # 1024 (cache row width, f32)
SCALE = HEAD_DIM ** -0.5
N_CORES = 8
GROUP = NUM_HEADS // NUM_KV_HEADS  # 4

SLOTS_PER_CORE = NUM_SLOTS // N_CORES   # 4096
RMAX = 4                                # tile rows (layout unchanged); chunks
# stream at r<=2 so arrival granularity stays at the compute cadence
RS = [1, 1] + [2] * 14 + [1, 1]
assert sum(RS) == SLOTS_PER_CORE // 128
NT = SLOTS_PER_CORE // 128              # 32 streamed 128-slot groups
QCOLS = NUM_KV_HEADS * BS * GROUP       # 512 score columns: (h, b, g)
H = NUM_KV_HEADS
LOGBIG = -1.2e6                         # ln-weight "-inf" (pre-divided by SCALE)


def build_program(mask_4d=True):
    import concourse.bacc as bacc
    import concourse.mybir as mybir
    import concourse.tile as tile

    f32 = mybir.dt.float32
    f32r = mybir.dt.float32r
    bf16 = mybir.dt.bfloat16
    EXP = mybir.ActivationFunctionType.Exp
    MULT = mybir.AluOpType.mult

    nc = bacc.Bacc("TRN2", target_bir_lowering=False, debug=False,
                   enable_asserts=False, num_devices=N_CORES,
                   num_swdge_queues=1)

    kc = nc.dram_tensor("k_shard", [SLOTS_PER_CORE, D], f32r,
                        kind="ExternalInput").ap()
    vc = nc.dram_tensor("v_shard", [SLOTS_PER_CORE, D], f32r,
                        kind="ExternalInput").ap()
    # host-transposed new-token K: [d, (h, j)]
    knewT_d = nc.dram_tensor("k_newT", [HEAD_DIM, H * BS], bf16,
                             kind="ExternalInput").ap()
    # new-token V (+ones col): [j, h*129]
    vnew_d = nc.dram_tensor("v_new16", [BS, H * 129], bf16,
                            kind="ExternalInput").ap()
    qT_d = nc.dram_tensor("qT", [HEAD_DIM, QCOLS], bf16,
                          kind="ExternalInput").ap()
    mask_d = nc.dram_tensor("mask", [128, NT * BS], bf16,
                            kind="ExternalInput").ap()
    # new-token ln-weights [b, j] and one-hot seq indicator [b, (h,b,g)]
    logmN_d = nc.dram_tensor("logmN", [BS, BS], bf16,
                             kind="ExternalInput").ap()
    ind_d = nc.dram_tensor("ind", [BS, QCOLS], bf16,
                           kind="ExternalInput").ap()
    ident_d = nc.dram_tensor("ident", [128, 128], f32r,
                             kind="ExternalInput").ap()
    # num packs PV and den: per head 129 cols = 128 d + 1 denominator
    num_d = nc.dram_tensor("num", [64, H * 129], f32,
                           kind="ExternalOutput").ap()

    chunks = []
    off = 0
    for r in RS:
        chunks.append((off, r))
        off += 128 * r

    with tile.TileContext(nc) as tc:
        with tc.tile_pool(name="const", bufs=1) as constp, \
             tc.tile_pool(name="kbuf", bufs=4) as kpool, \
             tc.tile_pool(name="vbuf", bufs=4) as vpool, \
             tc.tile_pool(name="ktsb", bufs=3) as ktp, \
             tc.tile_pool(name="psb", bufs=6) as ppool, \
             tc.tile_pool(name="outs", bufs=1) as outp, \
             tc.tile_pool(name="ps_kt", bufs=3, space="PSUM") as ps_kt, \
             tc.tile_pool(name="ps_s", bufs=2, space="PSUM") as ps_s, \
             tc.tile_pool(name="ps_pv", bufs=1, space="PSUM") as ps_pv:

            qt_sb = constp.tile([128, QCOLS], bf16)
            mask_sb = constp.tile([128, NT * BS], bf16)
            logmN_sb = constp.tile([BS, BS], bf16)
            ind_sb = constp.tile([BS, QCOLS], bf16)
            ident = constp.tile([128, 128], f32r)
            knewT = constp.tile([128, H * BS], bf16)
            vnew16 = constp.tile([BS, H, 129], bf16)

            # v16 rotation is persistent so the appended ones-columns
            # (denominator producers) are initialized exactly once
            v16 = [constp.tile([128, H, 129], bf16, name=f"v16_{i}")
                   for i in range(5)]
            for i in range(5):
                nc.vector.memset(v16[i][:, :, 128], 1.0)

            # pv[bg, h, 0:128] = numerator, pv[bg, h, 128] = denominator.
            # 129*4B per head: 3 heads fit a 2KB PSUM bank
            pvs = [ps_pv.tile([64, 3, 129], f32, name="pvA"),
                   ps_pv.tile([64, 3, 129], f32, name="pvB"),
                   ps_pv.tile([64, 2, 129], f32, name="pvC")]
            for t_ in pvs:
                nc.vector.memset(t_[:], 0.0)

            def pv_out(h):
                return pvs[h // 3][:, h % 3, :]

            def conv_v(vbuf, j, t):
                vv = v16[t % 5]
                src_v = vbuf[:, j, :].rearrange("p (h d) -> p h d", d=128)
                nc.scalar.copy(vv[:, 0:4, 0:128], src_v[:, 0:4, :])
                nc.vector.tensor_copy(vv[:, 4:8, 0:128], src_v[:, 4:8, :])
                return vv

            def transposes(ksrc):
                """K^T for one group, straight from f32r; the PSUM->SBUF
                copies do the bf16 cast."""
                ktsb = ktp.tile([128, H, 128], bf16, tag="kt")
                for hg in range(2):
                    ktps = ps_kt.tile([128, 4, 128], f32r, tag="ktps")
                    for i in range(4):
                        h = hg * 4 + i
                        nc.tensor.transpose(
                            ktps[:, i, :],
                            ksrc[:, h * 128:(h + 1) * 128],
                            ident[:])
                    dst = ktsb[:, hg * 4:hg * 4 + 4, :]
                    if hg == 0:
                        nc.vector.tensor_copy(dst, ktps[:])
                    else:
                        nc.scalar.copy(dst, ktps[:])
                return ktsb

            def qk_front(ktsb, t):
                # scores[slot, (h,b,g)] = sum_d K[slot,d_h] Q[(b,g),d]
                scores = ps_s.tile([128, QCOLS], f32, tag="scores")
                for h in range(H):
                    nc.tensor.matmul(
                        out=scores[:, h * 64:(h + 1) * 64],
                        lhsT=ktsb[:, h, :],
                        rhs=qt_sb[:, h * 64:(h + 1) * 64],
                        start=True, stop=True)

                p_sb = ppool.tile([128, QCOLS], bf16, tag="p")
                nc.scalar.activation(p_sb[:], scores[:], EXP, scale=SCALE)

                # ownership weights: pm[slot, (h,b,g)] = p * mask[slot, b]
                pm = ppool.tile([128, QCOLS], bf16, tag="pm")
                mcols = mask_sb[:, t * BS:(t + 1) * BS]
                if mask_4d:
                    nc.vector.tensor_tensor(
                        out=pm[:].rearrange("p (h b g) -> p h b g",
                                            h=H, b=BS, g=GROUP),
                        in0=p_sb[:].rearrange("p (h b g) -> p h b g",
                                              h=H, b=BS, g=GROUP),
                        in1=mcols.unsqueeze(1).unsqueeze(3).broadcast_to(
                            [128, H, BS, GROUP]),
                        op=MULT)
                else:
                    for h in range(H):
                        nc.vector.tensor_tensor(
                            out=pm[:, h * 64:(h + 1) * 64].rearrange(
                                "p (b g) -> p b g", g=GROUP),
                            in0=p_sb[:, h * 64:(h + 1) * 64].rearrange(
                                "p (b g) -> p b g", g=GROUP),
                            in1=mcols.unsqueeze(2).broadcast_to(
                                [128, BS, GROUP]),
                            op=MULT)

                return pm

            onum = outp.tile([64, H, 129], f32)

            def group_pv(pm, vv, last):
                # num[bg, h, d] += sum_slot pm[slot, bg] [V | 1][slot, d]
                for h in range(H):
                    nc.tensor.matmul(
                        out=pv_out(h),
                        lhsT=pm[:, h * 64:(h + 1) * 64],
                        rhs=vv[:, h, :],
                        start=False, stop=False, skip_group_check=True)
                    if last:
                        if h % 2 == 0:
                            nc.vector.tensor_copy(onum[:, h, :], pv_out(h))
                        else:
                            nc.scalar.copy(onum[:, h, :], pv_out(h))
                        if h == 5:
                            nc.sync.dma_start(
                                num_d[:, 0:6 * 129],
                                onum[:, 0:6, :].rearrange(
                                    "p h d -> p (h d)"))
                        elif h == 7:
                            nc.sync.dma_start(
                                num_d[:, 6 * 129:],
                                onum[:, 6:8, :].rearrange(
                                    "p h d -> p (h d)"))

            def new_token_group():
                """16-partition group for the scattered new-token k/v rows:
                host-pretransposed K, ln-weight mask folded into the score
                PSUM via a one-hot matmul, no transposes."""
                scoresN = ps_s.tile([BS, QCOLS], f32, tag="scores")
                nc.tensor.matmul(out=scoresN[:], lhsT=logmN_sb[:],
                                 rhs=ind_sb[:],
                                 start=True, stop=False,
                                 skip_group_check=True)
                for h in range(H):
                    nc.tensor.matmul(
                        out=scoresN[:, h * 64:(h + 1) * 64],
                        lhsT=knewT[:, h * BS:(h + 1) * BS],
                        rhs=qt_sb[:, h * 64:(h + 1) * 64],
                        start=False, stop=(h == H - 1),
                        skip_group_check=True)
                pmN = ppool.tile([BS, QCOLS], bf16, tag="pm")
                nc.scalar.activation(pmN[:], scoresN[:], EXP, scale=SCALE)
                for h in range(H):
                    nc.tensor.matmul(
                        out=pv_out(h),
                        lhsT=pmN[:, h * 64:(h + 1) * 64],
                        rhs=vnew16[:, h, :],
                        start=False, stop=False, skip_group_check=True)

            # software pipeline: V conversions run VLEAD groups ahead of
            # compute (emitted after each group's ops so exp/mask never queue
            # behind a conversion waiting on DMA); K needs no conversion
            VLEAD = 2
            chunk_iter = iter(chunks)
            avail = []          # (kbuf, vbuf, j) per group
            ready_v = {}        # t -> vv

            first_pump = [True]

            def pump():
                off, r = next(chunk_iter)
                kbuf = kpool.tile([128, RMAX, D], f32r, tag="k")
                vbuf = vpool.tile([128, RMAX, D], f32r, tag="v")
                ksrc = kc[off:off + 128 * r, :].rearrange(
                    "(p j) d -> p j d", j=r)
                vsrc = vc[off:off + 128 * r, :].rearrange(
                    "(p j) d -> p j d", j=r)
                nc.sync.dma_start(
                    kbuf[:, 0:r, :].rearrange("p j d -> p (j d)"),
                    ksrc.rearrange("p j d -> p (j d)"))
                if first_pump[0]:
                    nc.sync.dma_start(ident[:], ident_d)
                    nc.sync.dma_start(qt_sb[:], qT_d)
                nc.sync.dma_start(
                    vbuf[:, 0:r, :].rearrange("p j d -> p (j d)"),
                    vsrc.rearrange("p j d -> p (j d)"))
                if first_pump[0]:
                    first_pump[0] = False
                    nc.gpsimd.dma_start(mask_sb[:], mask_d)
                    nc.gpsimd.dma_start(knewT[:], knewT_d)
                    nc.gpsimd.dma_start(
                        vnew16[:].rearrange("p h d -> p (h d)"), vnew_d)
                    nc.gpsimd.dma_start(logmN_sb[:], logmN_d)
                    nc.gpsimd.dma_start(ind_sb[:], ind_d)
                avail.extend((kbuf, vbuf, j) for j in range(r))
                return kbuf, vbuf

            def ensure(upto_v):
                while n_v[0] <= min(upto_v, NT - 1):
                    tvv = n_v[0]
                    while len(avail) <= tvv:
                        pump()
                    ready_v[tvv] = conv_v(avail[tvv][1],
                                          avail[tvv][2], tvv)
                    n_v[0] += 1

            n_v = [0]
            pms = {}
            kts = {}
            ensure(VLEAD - 1)

            def ksrc_of(t):
                while len(avail) <= t:
                    pump()
                kb, _, jj = avail[t]
                return kb[:, jj, :]

            kts[0] = transposes(ksrc_of(0))
            for t in range(NT):
                pms[t] = qk_front(kts.pop(t), t)
                if t + 1 < NT:
                    kts[t + 1] = transposes(ksrc_of(t + 1))
                if t > 0:
                    group_pv(pms.pop(t - 1), ready_v.pop(t - 1), False)
                if t == 2:
                    # constants have landed by now; slot the new-token
                    # group in without stalling the tensor engine
                    new_token_group()
                ensure(t + VLEAD)
            group_pv(pms.pop(NT - 1), ready_v.pop(NT - 1), True)

    nc.compile()
    return nc


def shard_inputs(q, k, v, k_cache, v_cache, slot_mapping, page_indices):
    """Host-side sharding: contiguous zero-copy cache slices per core plus
    small index-derived tensors (masks, transposed queries/new tokens)."""
    q = np.ascontiguousarray(np.asarray(q, dtype=np.float32))
    k = np.ascontiguousarray(np.asarray(k, dtype=np.float32))
    v = np.ascontiguousarray(np.asarray(v, dtype=np.float32))
    k_cache = np.asarray(k_cache, dtype=np.float32)
    v_cache = np.asarray(v_cache, dtype=np.float32)
    slot_mapping = np.asarray(slot_mapping, dtype=np.int64).ravel()
    page_indices = np.asarray(page_indices, dtype=np.int64)

    # qT[d, (h, b, g)] = q[b, h*GROUP+g, d]
    qr = q.reshape(BS, NUM_KV_HEADS, GROUP, HEAD_DIM)
    qT = np.ascontiguousarray(
        qr.transpose(3, 1, 0, 2).reshape(HEAD_DIM, QCOLS)
    ).astype(ml_dtypes.bfloat16)

    # ownership weights: count[slot, b] = multiplicity of slot in seq b's pages
    count = np.zeros((NUM_SLOTS, BS), dtype=np.float32)
    np.add.at(count,
              (page_indices.ravel(),
               np.repeat(np.arange(BS), KV_LEN)),
              1.0)
    # new-token slots: reference scatters k/v rows there BEFORE the gather, so
    # the streamed (old) row must contribute nothing; the new-token group
    # re-injects each referencing (seq, count) exactly once fleet-wide.
    # With duplicate slot_mapping entries the last writer wins (jax .at[].set).
    final_writer = {}
    for j in range(BS):
        final_writer[int(slot_mapping[j])] = j
    newcnt = np.zeros((BS, BS), dtype=np.float32)   # [new-token row j, seq b]
    for s, j in final_writer.items():
        newcnt[j, :] = count[s, :]
        count[s, :] = 0.0

    # new-token ln-weights (pre-divided by SCALE so
    # exp(SCALE*(s+w)) = exp(SCALE*s)*cnt); streamed groups use plain counts
    lognew = np.where(newcnt > 0,
                      np.log(np.maximum(newcnt, 1e-30)) / SCALE,
                      LOGBIG).astype(np.float32)            # [j, b]

    # new-token tiles, host-converted to the device layouts
    knewT = np.ascontiguousarray(
        k.reshape(BS, H, HEAD_DIM).transpose(2, 1, 0).reshape(HEAD_DIM, H * BS)
    ).astype(ml_dtypes.bfloat16)                            # [d, (h, j)]
    vnew16 = np.zeros((BS, H, 129), dtype=np.float32)
    vnew16[:, :, 0:128] = v.reshape(BS, H, HEAD_DIM)
    vnew16[:, :, 128] = 1.0
    vnew16 = np.ascontiguousarray(
        vnew16.reshape(BS, H * 129)).astype(ml_dtypes.bfloat16)

    # one-hot seq indicator [b', (h, b, g)]
    ind = np.zeros((BS, H, BS, GROUP), dtype=np.float32)
    ind[np.arange(BS), :, np.arange(BS), :] = 1.0
    ind = np.ascontiguousarray(ind.reshape(BS, QCOLS)).astype(
        ml_dtypes.bfloat16)

    # streamed slot (group t from chunk (off, r), partition p, sub j)
    #   = off + p*r + j ; group index t advances j-major within a chunk
    perm = np.empty(SLOTS_PER_CORE, dtype=np.int64)
    gi = 0
    off = 0
    for r in RS:
        idx = off + np.arange(128)[:, None] * r + np.arange(r)[None, :]
        for j in range(r):
            perm[gi * 128:(gi + 1) * 128] = idx[:, j]
            gi += 1
        off += 128 * r

    in_maps = []
    for c in range(N_CORES):
        base = c * SLOTS_PER_CORE
        mcore = count[base:base + SLOTS_PER_CORE][perm]     # [4096, 16]
        m = (mcore.reshape(NT, 128, BS).transpose(1, 0, 2)
             .reshape(128, NT * BS))
        # new-token rows: row j handled by core j % N_CORES
        lm_new = np.full((BS, BS), LOGBIG, dtype=np.float32)
        for j in range(BS):
            if j % N_CORES == c:
                lm_new[:, j] = lognew[j]
        in_maps.append({
            "k_shard": k_cache[base:base + SLOTS_PER_CORE],
            "v_shard": v_cache[base:base + SLOTS_PER_CORE],
            "k_newT": knewT,
            "v_new16": vnew16,
            "qT": qT,
            "mask": np.ascontiguousarray(m).astype(ml_dtypes.bfloat16),
            "logmN": lm_new.astype(ml_dtypes.bfloat16),
            "ind": ind,
            "ident": np.eye(128, dtype=np.float32),
        })
    return in_maps


_PROGS = {}
last_results = None  # BassKernelResults of the most recent kernel() call


def kernel(q, k, v, k_cache, v_cache, slot_mapping, page_indices):
    global last_results
    from concourse.bass_utils import run_bass_kernel_spmd

    in_maps = shard_inputs(q, k, v, k_cache, v_cache, slot_mapping,
                           page_indices)
    if "prog" not in _PROGS:
        try:
            _PROGS["prog"] = build_program(mask_4d=True)
        except Exception:
            _PROGS["prog"] = build_program(mask_4d=False)
    res = run_bass_kernel_spmd(_PROGS["prog"], in_maps,
                               core_ids=list(range(N_CORES)))
    last_results = res

    acc = np.zeros((64, H, 129), dtype=np.float64)
    for c in range(N_CORES):
        acc += res.results[c]["num"].astype(np.float64).reshape(64, H, 129)
    num = acc[:, :, 0:128]                      # [(b,g), h]
    den = acc[:, :, 128]                        # [(b,g), h]
    o = num / den[:, :, None]
    o = o.reshape(BS, GROUP, NUM_KV_HEADS, HEAD_DIM)   # [b, g, h, d]
    out = o.transpose(0, 2, 1, 3).reshape(BS, NUM_HEADS * HEAD_DIM)
    return np.ascontiguousarray(out.astype(np.float32))


# revision 25
# speedup vs baseline: 1.0091x; 1.0091x over previous
"""Paged GQA decode attention (sparse_attention) on 8 TRN2 NeuronCores.

Slot-parallel streaming design: the page table is (in the graded problem) a
permutation of all 32768 cache slots, so the full K/V caches are read exactly
once.  Core i owns the contiguous slot range [i*4096, (i+1)*4096) and STREAMS
it sequentially at the ~358 GB/s per-core HBM roofline on the SP HWDGE queue,
in chunks of r*128 slots ([128 partitions, r rows, 1024]); r<=2 keeps chunk
arrival granularity at or below the per-group compute cadence so the PE never
sits idle waiting for a large chunk to land.

Each core scores its slots against ALL 16 sequences' queries, multiplies by a
host-built per-(slot, seq) ownership weight (0/1 under the permutation;
counts for general inputs), and accumulates the partial attention numerator
and denominator in PSUM over the whole stream.  The 16 new-token k/v rows are
host-transposed (knewT) and processed as a cheap 16-partition group with NO
on-device transposes, emitted mid-pipeline (after streamed group 2) so the
tensor engine never stalls waiting for its constants; its ownership weights
arrive as ln-weights accumulated into the score PSUM through a one-hot
[16,j]^T @ [16,qcols] matmul so Exp alone masks foreign rows.  The streamed
ownership weight of a replaced slot is zeroed on the core that owns it.  No
gathers, no scatters -- one fully static program per core.  The host sums the
per-core partials (flash-decoding split-K combine) and normalizes.

Per-instruction HW profiling showed the tensor engine is cadence-bound at
roughly max(ldweights, matmul-stream) ~ 1ns/column, and Vector/Scalar
instructions cost ~690ns nearly independent of size, so the pipeline
minimizes PE columns moved and DVE/ACT instruction count:
  - everything on the PE is bf16: K tiles are converted f32->bf16 via the
    transpose PSUM->SBUF copies (one on Vector, one on Scalar) and V tiles
    by one Scalar + one Vector copy, two groups ahead of their consumers,
  - PV is flipped (lhsT = masked-P, 64-column weight loads; V is the moving
    operand) and V carries an appended ones-column per head so each PV
    matmul also emits that head's denominator -- no separate den matmul,
  - PSUM accumulators are memset once and PV runs start=stop=False
    (skip_group_check): TRN2 zeroes lazily per 2KB region on
    start_tensor_calc, so 8 per-head groups in one bank would clobber each
    other.
The output numerator is copied and DMA'd out in two halves so the second
half's transfer latency is all that remains after the last PV.
"""

import ml_dtypes
import numpy as np

# ---- problem constants (must match the harness's reference.py) ----
NUM_HEADS = 32
NUM_KV_HEADS = 8
HEAD_DIM = 128
BS = 16
KV_LEN = 2048
NUM_SLOTS = BS * KV_LEN          # 32768
D = NUM_KV_HEADS * HEAD_DIM      # 1024 (cache row width, f32)
SCALE = HEAD_DIM ** -0.5
N_CORES = 8
GROUP = NUM_HEADS // NUM_KV_HEADS  # 4

SLOTS_PER_CORE = NUM_SLOTS // N_CORES   # 4096
RMAX = 4                                # max DRAM rows per partition per chunk
# per-chunk rows/partition: small chunks at both ends for pipeline fill/drain;
# big mid-stream chunks amortize HWDGE descriptor-transition overhead (r=2
# everywhere measured ~15% LOWER effective DMA bandwidth)
RS = [1, 1, 2, 4, 4, 4, 4, 4, 4, 2, 1, 1]
assert sum(RS) == SLOTS_PER_CORE // 128
NT = SLOTS_PER_CORE // 128              # 32 streamed 128-slot groups
QCOLS = NUM_KV_HEADS * BS * GROUP       # 512 score columns: (h, b, g)
H = NUM_KV_HEADS
LOGBIG = -1.2e6                         # ln-weight "-inf" (pre-divided by SCALE)


def build_program(mask_4d=True):
    import concourse.bacc as bacc
    import concourse.mybir as mybir
    import concourse.tile as tile

    f32 = mybir.dt.float32
    f32r = mybir.dt.float32r
    bf16 = mybir.dt.bfloat16
    EXP = mybir.ActivationFunctionType.Exp
    MULT = mybir.AluOpType.mult

    nc = bacc.Bacc("TRN2", target_bir_lowering=False, debug=False,
                   enable_asserts=False, num_devices=N_CORES,
                   num_swdge_queues=1)

    kc = nc.dram_tensor("k_shard", [SLOTS_PER_CORE, D], f32r,
                        kind="ExternalInput").ap()
    vc = nc.dram_tensor("v_shard", [SLOTS_PER_CORE, D], f32r,
                        kind="ExternalInput").ap()
    # host-transposed new-token K: [d, (h, j)]
    knewT_d = nc.dram_tensor("k_newT", [HEAD_DIM, H * BS], bf16,
                             kind="ExternalInput").ap()
    # new-token V (+ones col): [j, h*129]
    vnew_d = nc.dram_tensor("v_new16", [BS, H * 129], bf16,
                            kind="ExternalInput").ap()
    qT_d = nc.dram_tensor("qT", [HEAD_DIM, QCOLS], bf16,
                          kind="ExternalInput").ap()
    mask_d = nc.dram_tensor("mask", [128, NT * BS], bf16,
                            kind="ExternalInput").ap()
    # new-token ln-weights [b, j] and one-hot seq indicator [b, (h,b,g)]
    logmN_d = nc.dram_tensor("logmN", [BS, BS], bf16,
                             kind="ExternalInput").ap()
    ind_d = nc.dram_tensor("ind", [BS, QCOLS], bf16,
                           kind="ExternalInput").ap()
    ident_d = nc.dram_tensor("ident", [128, 128], f32r,
                             kind="ExternalInput").ap()
    # num packs PV and den: per head 129 cols = 128 d + 1 denominator
    num_d = nc.dram_tensor("num", [64, H * 129], f32,
                           kind="ExternalOutput").ap()

    chunks = []
    off = 0
    for r in RS:
        chunks.append((off, r))
        off += 128 * r

    with tile.TileContext(nc) as tc:
        with tc.tile_pool(name="const", bufs=1) as constp, \
             tc.tile_pool(name="kbuf", bufs=4) as kpool, \
             tc.tile_pool(name="vbuf", bufs=4) as vpool, \
             tc.tile_pool(name="ktsb", bufs=3) as ktp, \
             tc.tile_pool(name="psb", bufs=6) as ppool, \
             tc.tile_pool(name="outs", bufs=1) as outp, \
             tc.tile_pool(name="ps_kt", bufs=3, space="PSUM") as ps_kt, \
             tc.tile_pool(name="ps_s", bufs=2, space="PSUM") as ps_s, \
             tc.tile_pool(name="ps_pv", bufs=1, space="PSUM") as ps_pv:

            qt_sb = constp.tile([128, QCOLS], bf16)
            mask_sb = constp.tile([128, NT * BS], bf16)
            logmN_sb = constp.tile([BS, BS], bf16)
            ind_sb = constp.tile([BS, QCOLS], bf16)
            ident = constp.tile([128, 128], f32r)
            knewT = constp.tile([128, H * BS], bf16)
            vnew16 = constp.tile([BS, H, 129], bf16)

            # v16 rotation is persistent so the appended ones-columns
            # (denominator producers) are initialized exactly once
            v16 = [constp.tile([128, H, 129], bf16, name=f"v16_{i}")
                   for i in range(5)]
            for i in range(5):
                nc.vector.memset(v16[i][:, :, 128], 1.0)

            # pv[bg, h, 0:128] = numerator, pv[bg, h, 128] = denominator.
            # 129*4B per head: 3 heads fit a 2KB PSUM bank
            pvs = [ps_pv.tile([64, 3, 129], f32, name="pvA"),
                   ps_pv.tile([64, 3, 129], f32, name="pvB"),
                   ps_pv.tile([64, 2, 129], f32, name="pvC")]
            for t_ in pvs:
                nc.vector.memset(t_[:], 0.0)

            def pv_out(h):
                return pvs[h // 3][:, h % 3, :]

            def conv_v(vbuf, j, t):
                vv = v16[t % 5]
                src_v = vbuf[:, j, :].rearrange("p (h d) -> p h d", d=128)
                nc.scalar.copy(vv[:, 0:4, 0:128], src_v[:, 0:4, :])
                nc.vector.tensor_copy(vv[:, 4:8, 0:128], src_v[:, 4:8, :])
                return vv

            def transposes(ksrc):
                """K^T for one group, straight from f32r; the PSUM->SBUF
                copies do the bf16 cast."""
                ktsb = ktp.tile([128, H, 128], bf16, tag="kt")
                for hg in range(2):
                    ktps = ps_kt.tile([128, 4, 128], f32r, tag="ktps")
                    for i in range(4):
                        h = hg * 4 + i
                        nc.tensor.transpose(
                            ktps[:, i, :],
                            ksrc[:, h * 128:(h + 1) * 128],
                            ident[:])
                    dst = ktsb[:, hg * 4:hg * 4 + 4, :]
                    if hg == 0:
                        nc.vector.tensor_copy(dst, ktps[:])
                    else:
                        nc.scalar.copy(dst, ktps[:])
                return ktsb

            def qk_front(ktsb, t):
                # scores[slot, (h,b,g)] = sum_d K[slot,d_h] Q[(b,g),d]
                scores = ps_s.tile([128, QCOLS], f32, tag="scores")
                for h in range(H):
                    nc.tensor.matmul(
                        out=scores[:, h * 64:(h + 1) * 64],
                        lhsT=ktsb[:, h, :],
                        rhs=qt_sb[:, h * 64:(h + 1) * 64],
                        start=True, stop=True)

                p_sb = ppool.tile([128, QCOLS], bf16, tag="p")
                nc.scalar.activation(p_sb[:], scores[:], EXP, scale=SCALE)

                # ownership weights: pm[slot, (h,b,g)] = p * mask[slot, b]
                pm = ppool.tile([128, QCOLS], bf16, tag="pm")
                mcols = mask_sb[:, t * BS:(t + 1) * BS]
                if mask_4d:
                    nc.vector.tensor_tensor(
                        out=pm[:].rearrange("p (h b g) -> p h b g",
                                            h=H, b=BS, g=GROUP),
                        in0=p_sb[:].rearrange("p (h b g) -> p h b g",
                                              h=H, b=BS, g=GROUP),
                        in1=mcols.unsqueeze(1).unsqueeze(3).broadcast_to(
                            [128, H, BS, GROUP]),
                        op=MULT)
                else:
                    for h in range(H):
                        nc.vector.tensor_tensor(
                            out=pm[:, h * 64:(h + 1) * 64].rearrange(
                                "p (b g) -> p b g", g=GROUP),
                            in0=p_sb[:, h * 64:(h + 1) * 64].rearrange(
                                "p (b g) -> p b g", g=GROUP),
                            in1=mcols.unsqueeze(2).broadcast_to(
                                [128, BS, GROUP]),
                            op=MULT)

                return pm

            onum = outp.tile([64, H, 129], f32)

            def group_pv(pm, vv, last):
                # num[bg, h, d] += sum_slot pm[slot, bg] [V | 1][slot, d]
                for h in range(H):
                    nc.tensor.matmul(
                        out=pv_out(h),
                        lhsT=pm[:, h * 64:(h + 1) * 64],
                        rhs=vv[:, h, :],
                        start=False, stop=False, skip_group_check=True)
                    if last:
                        if h % 2 == 0:
                            nc.vector.tensor_copy(onum[:, h, :], pv_out(h))
                        else:
                            nc.scalar.copy(onum[:, h, :], pv_out(h))
                        if h == 5:
                            nc.sync.dma_start(
                                num_d[:, 0:6 * 129],
                                onum[:, 0:6, :].rearrange(
                                    "p h d -> p (h d)"))
                        elif h == 7:
                            nc.sync.dma_start(
                                num_d[:, 6 * 129:],
                                onum[:, 6:8, :].rearrange(
                                    "p h d -> p (h d)"))

            def new_token_group():
                """16-partition group for the scattered new-token k/v rows:
                host-pretransposed K, ln-weight mask folded into the score
                PSUM via a one-hot matmul, no transposes."""
                scoresN = ps_s.tile([BS, QCOLS], f32, tag="scores")
                nc.tensor.matmul(out=scoresN[:], lhsT=logmN_sb[:],
                                 rhs=ind_sb[:],
                                 start=True, stop=False,
                                 skip_group_check=True)
                for h in range(H):
                    nc.tensor.matmul(
                        out=scoresN[:, h * 64:(h + 1) * 64],
                        lhsT=knewT[:, h * BS:(h + 1) * BS],
                        rhs=qt_sb[:, h * 64:(h + 1) * 64],
                        start=False, stop=(h == H - 1),
                        skip_group_check=True)
                pmN = ppool.tile([BS, QCOLS], bf16, tag="pm")
                nc.scalar.activation(pmN[:], scoresN[:], EXP, scale=SCALE)
                for h in range(H):
                    nc.tensor.matmul(
                        out=pv_out(h),
                        lhsT=pmN[:, h * 64:(h + 1) * 64],
                        rhs=vnew16[:, h, :],
                        start=False, stop=False, skip_group_check=True)

            # software pipeline: V conversions run VLEAD groups ahead of
            # compute (emitted after each group's ops so exp/mask never queue
            # behind a conversion waiting on DMA); K needs no conversion
            VLEAD = 2
            chunk_iter = iter(chunks)
            avail = []          # (kbuf, vbuf, j) per group
            ready_v = {}        # t -> vv

            first_pump = [True]

            def pump():
                off, r = next(chunk_iter)
                kbuf = kpool.tile([128, RMAX, D], f32r, tag="k")
                vbuf = vpool.tile([128, RMAX, D], f32r, tag="v")
                ksrc = kc[off:off + 128 * r, :].rearrange(
                    "(p j) d -> p j d", j=r)
                vsrc = vc[off:off + 128 * r, :].rearrange(
                    "(p j) d -> p j d", j=r)
                if first_pump[0]:
                    # tiny ident first: transposes(0) unblock right after K0
                    nc.sync.dma_start(ident[:], ident_d)
                nc.sync.dma_start(
                    kbuf[:, 0:r, :].rearrange("p j d -> p (j d)"),
                    ksrc.rearrange("p j d -> p (j d)"))
                if first_pump[0]:
                    nc.sync.dma_start(qt_sb[:], qT_d)
                nc.sync.dma_start(
                    vbuf[:, 0:r, :].rearrange("p j d -> p (j d)"),
                    vsrc.rearrange("p j d -> p (j d)"))
                if first_pump[0]:
                    first_pump[0] = False
                    nc.sync.dma_start(mask_sb[:], mask_d)
                    nc.sync.dma_start(knewT[:], knewT_d)
                    nc.sync.dma_start(
                        vnew16[:].rearrange("p h d -> p (h d)"), vnew_d)
                    nc.sync.dma_start(logmN_sb[:], logmN_d)
                    nc.sync.dma_start(ind_sb[:], ind_d)
                avail.extend((kbuf, vbuf, j) for j in range(r))
                return kbuf, vbuf

            def ensure(upto_v):
                while n_v[0] <= min(upto_v, NT - 1):
                    tvv = n_v[0]
                    while len(avail) <= tvv:
                        pump()
                    ready_v[tvv] = conv_v(avail[tvv][1],
                                          avail[tvv][2], tvv)
                    n_v[0] += 1

            n_v = [0]
            pms = {}
            kts = {}
            ensure(VLEAD - 1)

            def ksrc_of(t):
                while len(avail) <= t:
                    pump()
                kb, _, jj = avail[t]
                return kb[:, jj, :]

            kts[0] = transposes(ksrc_of(0))
            for t in range(NT):
                pms[t] = qk_front(kts.pop(t), t)
                if t + 1 < NT:
                    kts[t + 1] = transposes(ksrc_of(t + 1))
                if t > 0:
                    group_pv(pms.pop(t - 1), ready_v.pop(t - 1), False)
                if t == 2:
                    # constants have landed by now; slot the new-token
                    # group in without stalling the tensor engine
                    new_token_group()
                ensure(t + VLEAD)
            group_pv(pms.pop(NT - 1), ready_v.pop(NT - 1), True)

    nc.compile()
    return nc


def shard_inputs(q, k, v, k_cache, v_cache, slot_mapping, page_indices):
    """Host-side sharding: contiguous zero-copy cache slices per core plus
    small index-derived tensors (masks, transposed queries/new tokens)."""
    q = np.ascontiguousarray(np.asarray(q, dtype=np.float32))
    k = np.ascontiguousarray(np.asarray(k, dtype=np.float32))
    v = np.ascontiguousarray(np.asarray(v, dtype=np.float32))
    k_cache = np.asarray(k_cache, dtype=np.float32)
    v_cache = np.asarray(v_cache, dtype=np.float32)
    slot_mapping = np.asarray(slot_mapping, dtype=np.int64).ravel()
    page_indices = np.asarray(page_indices, dtype=np.int64)

    # qT[d, (h, b, g)] = q[b, h*GROUP+g, d]
    qr = q.reshape(BS, NUM_KV_HEADS, GROUP, HEAD_DIM)
    qT = np.ascontiguousarray(
        qr.transpose(3, 1, 0, 2).reshape(HEAD_DIM, QCOLS)
    ).astype(ml_dtypes.bfloat16)

    # ownership weights: count[slot, b] = multiplicity of slot in seq b's pages
    count = np.zeros((NUM_SLOTS, BS), dtype=np.float32)
    np.add.at(count,
              (page_indices.ravel(),
               np.repeat(np.arange(BS), KV_LEN)),
              1.0)
    # new-token slots: reference scatters k/v rows there BEFORE the gather, so
    # the streamed (old) row must contribute nothing; the new-token group
    # re-injects each referencing (seq, count) exactly once fleet-wide.
    # With duplicate slot_mapping entries the last writer wins (jax .at[].set).
    final_writer = {}
    for j in range(BS):
        final_writer[int(slot_mapping[j])] = j
    newcnt = np.zeros((BS, BS), dtype=np.float32)   # [new-token row j, seq b]
    for s, j in final_writer.items():
        newcnt[j, :] = count[s, :]
        count[s, :] = 0.0

    # new-token ln-weights (pre-divided by SCALE so
    # exp(SCALE*(s+w)) = exp(SCALE*s)*cnt); streamed groups use plain counts
    lognew = np.where(newcnt > 0,
                      np.log(np.maximum(newcnt, 1e-30)) / SCALE,
                      LOGBIG).astype(np.float32)            # [j, b]

    # new-token tiles, host-converted to the device layouts
    knewT = np.ascontiguousarray(
        k.reshape(BS, H, HEAD_DIM).transpose(2, 1, 0).reshape(HEAD_DIM, H * BS)
    ).astype(ml_dtypes.bfloat16)                            # [d, (h, j)]
    vnew16 = np.zeros((BS, H, 129), dtype=np.float32)
    vnew16[:, :, 0:128] = v.reshape(BS, H, HEAD_DIM)
    vnew16[:, :, 128] = 1.0
    vnew16 = np.ascontiguousarray(
        vnew16.reshape(BS, H * 129)).astype(ml_dtypes.bfloat16)

    # one-hot seq indicator [b', (h, b, g)]
    ind = np.zeros((BS, H, BS, GROUP), dtype=np.float32)
    ind[np.arange(BS), :, np.arange(BS), :] = 1.0
    ind = np.ascontiguousarray(ind.reshape(BS, QCOLS)).astype(
        ml_dtypes.bfloat16)

    # streamed slot (group t from chunk (off, r), partition p, sub j)
    #   = off + p*r + j ; group index t advances j-major within a chunk
    perm = np.empty(SLOTS_PER_CORE, dtype=np.int64)
    gi = 0
    off = 0
    for r in RS:
        idx = off + np.arange(128)[:, None] * r + np.arange(r)[None, :]
        for j in range(r):
            perm[gi * 128:(gi + 1) * 128] = idx[:, j]
            gi += 1
        off += 128 * r

    in_maps = []
    for c in range(N_CORES):
        base = c * SLOTS_PER_CORE
        mcore = count[base:base + SLOTS_PER_CORE][perm]     # [4096, 16]
        m = (mcore.reshape(NT, 128, BS).transpose(1, 0, 2)
             .reshape(128, NT * BS))
        # new-token rows: row j handled by core j % N_CORES
        lm_new = np.full((BS, BS), LOGBIG, dtype=np.float32)
        for j in range(BS):
            if j % N_CORES == c:
                lm_new[:, j] = lognew[j]
        in_maps.append({
            "k_shard": k_cache[base:base + SLOTS_PER_CORE],
            "v_shard": v_cache[base:base + SLOTS_PER_CORE],
            "k_newT": knewT,
            "v_new16": vnew16,
            "qT": qT,
            "mask": np.ascontiguousarray(m).astype(ml_dtypes.bfloat16),
            "logmN": lm_new.astype(ml_dtypes.bfloat16),
            "ind": ind,
            "ident": np.eye(128, dtype=np.float32),
        })
    return in_maps


_PROGS = {}
last_results = None  # BassKernelResults of the most recent kernel() call


def kernel(q, k, v, k_cache, v_cache, slot_mapping, page_indices):
    global last_results
    from concourse.bass_utils import run_bass_kernel_spmd

    in_maps = shard_inputs(q, k, v, k_cache, v_cache, slot_mapping,
                           page_indices)
    if "prog" not in _PROGS:
        try:
            _PROGS["prog"] = build_program(mask_4d=True)
        except Exception:
            _PROGS["prog"] = build_program(mask_4d=False)
    res = run_bass_kernel_spmd(_PROGS["prog"], in_maps,
                               core_ids=list(range(N_CORES)))
    last_results = res

    acc = np.zeros((64, H, 129), dtype=np.float64)
    for c in range(N_CORES):
        acc += res.results[c]["num"].astype(np.float64).reshape(64, H, 129)
    num = acc[:, :, 0:128]                      # [(b,g), h]
    den = acc[:, :, 128]                        # [(b,g), h]
    o = num / den[:, :, None]
    o = o.reshape(BS, GROUP, NUM_KV_HEADS, HEAD_DIM)   # [b, g, h, d]
    out = o.transpose(0, 2, 1, 3).reshape(BS, NUM_HEADS * HEAD_DIM)
    return np.ascontiguousarray(out.astype(np.float32))


# revision 26
# speedup vs baseline: 1.2044x; 1.1936x over previous
"""Paged GQA decode attention (sparse_attention) on 8 TRN2 NeuronCores.

Slot-parallel streaming design: the page table is (in the graded problem) a
permutation of all 32768 cache slots, so the full K/V caches are read exactly
once.  Core i owns the contiguous slot range [i*4096, (i+1)*4096) and STREAMS
it sequentially at the ~358 GB/s per-core HBM roofline on the SP HWDGE queue,
in chunks of r*128 slots ([128 partitions, r rows, 1024]); r<=2 keeps chunk
arrival granularity at or below the per-group compute cadence so the PE never
sits idle waiting for a large chunk to land.

Each core scores its slots against ALL 16 sequences' queries, multiplies by a
host-built per-(slot, seq) ownership weight (0/1 under the permutation;
counts for general inputs), and accumulates the partial attention numerator
and denominator in PSUM over the whole stream.  The 16 new-token k/v rows are
host-transposed (knewT) and processed as a cheap 16-partition group with NO
on-device transposes, emitted mid-pipeline (after streamed group 2) so the
tensor engine never stalls waiting for its constants; its ownership weights
arrive as ln-weights accumulated into the score PSUM through a one-hot
[16,j]^T @ [16,qcols] matmul so Exp alone masks foreign rows.  The streamed
ownership weight of a replaced slot is zeroed on the core that owns it.  No
gathers, no scatters -- one fully static program per core.  The host sums the
per-core partials (flash-decoding split-K combine) and normalizes.

Per-instruction HW profiling showed the tensor engine is cadence-bound at
roughly max(ldweights, matmul-stream) ~ 1ns/column, and Vector/Scalar
instructions cost ~690ns nearly independent of size, so the pipeline
minimizes PE columns moved and DVE/ACT instruction count:
  - everything on the PE is bf16: K tiles are converted f32->bf16 via the
    transpose PSUM->SBUF copies (one on Vector, one on Scalar) and V tiles
    by one Scalar + one Vector copy, two groups ahead of their consumers,
  - PV is flipped (lhsT = masked-P, 64-column weight loads; V is the moving
    operand) and V carries an appended ones-column per head so each PV
    matmul also emits that head's denominator -- no separate den matmul,
  - PSUM accumulators are memset once and PV runs start=stop=False
    (skip_group_check): TRN2 zeroes lazily per 2KB region on
    start_tensor_calc, so 8 per-head groups in one bank would clobber each
    other.
The output numerator is copied and DMA'd out in two halves so the second
half's transfer latency is all that remains after the last PV.
"""

import ml_dtypes
import numpy as np

# ---- problem constants (must match the harness's reference.py) ----
NUM_HEADS = 32
NUM_KV_HEADS = 8
HEAD_DIM = 128
BS = 16
KV_LEN = 2048
NUM_SLOTS = BS * KV_LEN          # 32768
D = NUM_KV_HEADS * HEAD_DIM      # 1024 (cache row width, f32)
SCALE = HEAD_DIM ** -0.5
N_CORES = 8
GROUP = NUM_HEADS // NUM_KV_HEADS  # 4

SLOTS_PER_CORE = NUM_SLOTS // N_CORES   # 4096
RMAX = 4                                # max DRAM rows per partition per chunk
# per-chunk rows/partition: small chunks at both ends for pipeline fill/drain;
# big mid-stream chunks amortize HWDGE descriptor-transition overhead (r=2
# everywhere measured ~15% LOWER effective DMA bandwidth)
RS = [1, 1, 2, 4, 4, 4, 4, 4, 4, 2, 1, 1]
assert sum(RS) == SLOTS_PER_CORE // 128
NT = SLOTS_PER_CORE // 128              # 32 streamed 128-slot groups
QCOLS = NUM_KV_HEADS * BS * GROUP       # 512 score columns: (h, b, g)
H = NUM_KV_HEADS
LOGBIG = -1.2e6                         # ln-weight "-inf" (pre-divided by SCALE)


def build_program(mask_4d=True):
    import concourse.bacc as bacc
    import concourse.mybir as mybir
    import concourse.tile as tile

    f32 = mybir.dt.float32
    f32r = mybir.dt.float32r
    bf16 = mybir.dt.bfloat16
    EXP = mybir.ActivationFunctionType.Exp
    MULT = mybir.AluOpType.mult

    nc = bacc.Bacc("TRN2", target_bir_lowering=False, debug=False,
                   enable_asserts=False, num_devices=N_CORES,
                   num_swdge_queues=1)

    kc = nc.dram_tensor("k_shard", [SLOTS_PER_CORE, D], f32r,
                        kind="ExternalInput").ap()
    vc = nc.dram_tensor("v_shard", [SLOTS_PER_CORE, D], f32r,
                        kind="ExternalInput").ap()
    # host-transposed new-token K: [d, (h, j)]
    knewT_d = nc.dram_tensor("k_newT", [HEAD_DIM, H * BS], bf16,
                             kind="ExternalInput").ap()
    # new-token V (+ones col): [j, h*129]
    vnew_d = nc.dram_tensor("v_new16", [BS, H * 129], bf16,
                            kind="ExternalInput").ap()
    qT_d = nc.dram_tensor("qT", [HEAD_DIM, QCOLS], bf16,
                          kind="ExternalInput").ap()
    mask_d = nc.dram_tensor("mask", [128, NT * BS], bf16,
                            kind="ExternalInput").ap()
    # new-token ln-weights [b, j] and one-hot seq indicator [b, (h,b,g)]
    logmN_d = nc.dram_tensor("logmN", [BS, BS], bf16,
                             kind="ExternalInput").ap()
    ind_d = nc.dram_tensor("ind", [BS, QCOLS], bf16,
                           kind="ExternalInput").ap()
    ident_d = nc.dram_tensor("ident", [128, 128], f32r,
                             kind="ExternalInput").ap()
    # num packs PV and den: per head 129 cols = 128 d + 1 denominator
    num_d = nc.dram_tensor("num", [64, H * 129], f32,
                           kind="ExternalOutput").ap()

    chunks = []
    off = 0
    for r in RS:
        chunks.append((off, r))
        off += 128 * r

    with tile.TileContext(nc) as tc:
        with tc.tile_pool(name="const", bufs=1) as constp, \
             tc.tile_pool(name="kbuf", bufs=4) as kpool, \
             tc.tile_pool(name="vbuf", bufs=4) as vpool, \
             tc.tile_pool(name="ktsb", bufs=3) as ktp, \
             tc.tile_pool(name="psb", bufs=6) as ppool, \
             tc.tile_pool(name="outs", bufs=1) as outp, \
             tc.tile_pool(name="ps_kt", bufs=3, space="PSUM") as ps_kt, \
             tc.tile_pool(name="ps_s", bufs=2, space="PSUM") as ps_s, \
             tc.tile_pool(name="ps_pv", bufs=1, space="PSUM") as ps_pv:

            qt_sb = constp.tile([128, QCOLS], bf16)
            mask_sb = constp.tile([128, NT * BS], bf16)
            logmN_sb = constp.tile([BS, BS], bf16)
            ind_sb = constp.tile([BS, QCOLS], bf16)
            ident = constp.tile([128, 128], f32r)
            knewT = constp.tile([128, H * BS], bf16)
            vnew16 = constp.tile([BS, H, 129], bf16)

            # v16 rotation is persistent so the appended ones-columns
            # (denominator producers) are initialized exactly once
            v16 = [constp.tile([128, H, 129], bf16, name=f"v16_{i}")
                   for i in range(5)]
            for i in range(5):
                nc.vector.memset(v16[i][:, :, 128], 1.0)

            # pv[bg, h, 0:128] = numerator, pv[bg, h, 128] = denominator.
            # 129*4B per head: 3 heads fit a 2KB PSUM bank
            pvs = [ps_pv.tile([64, 3, 129], f32, name="pvA"),
                   ps_pv.tile([64, 3, 129], f32, name="pvB"),
                   ps_pv.tile([64, 2, 129], f32, name="pvC")]
            for t_ in pvs:
                nc.vector.memset(t_[:], 0.0)

            def pv_out(h):
                return pvs[h // 3][:, h % 3, :]

            def conv_v(vbuf, j, t):
                vv = v16[t % 5]
                src_v = vbuf[:, j, :].rearrange("p (h d) -> p h d", d=128)
                nc.scalar.copy(vv[:, 0:4, 0:128], src_v[:, 0:4, :])
                nc.vector.tensor_copy(vv[:, 4:8, 0:128], src_v[:, 4:8, :])
                return vv

            def transposes(ksrc):
                """K^T for one group, straight from f32r; the PSUM->SBUF
                copies do the bf16 cast."""
                ktsb = ktp.tile([128, H, 128], bf16, tag="kt")
                for hg in range(2):
                    ktps = ps_kt.tile([128, 4, 128], f32r, tag="ktps")
                    for i in range(4):
                        h = hg * 4 + i
                        nc.tensor.transpose(
                            ktps[:, i, :],
                            ksrc[:, h * 128:(h + 1) * 128],
                            ident[:])
                    dst = ktsb[:, hg * 4:hg * 4 + 4, :]
                    if hg == 0:
                        nc.vector.tensor_copy(dst, ktps[:])
                    else:
                        nc.scalar.copy(dst, ktps[:])
                return ktsb

            def qk_front(ktsb, t):
                # scores[slot, (h,b,g)] = sum_d K[slot,d_h] Q[(b,g),d]
                scores = ps_s.tile([128, QCOLS], f32, tag="scores")
                for h in range(H):
                    nc.tensor.matmul(
                        out=scores[:, h * 64:(h + 1) * 64],
                        lhsT=ktsb[:, h, :],
                        rhs=qt_sb[:, h * 64:(h + 1) * 64],
                        start=True, stop=True)

                p_sb = ppool.tile([128, QCOLS], bf16, tag="p")
                nc.scalar.activation(p_sb[:], scores[:], EXP, scale=SCALE)

                # ownership weights: pm[slot, (h,b,g)] = p * mask[slot, b]
                pm = ppool.tile([128, QCOLS], bf16, tag="pm")
                mcols = mask_sb[:, t * BS:(t + 1) * BS]
                if mask_4d:
                    nc.vector.tensor_tensor(
                        out=pm[:].rearrange("p (h b g) -> p h b g",
                                            h=H, b=BS, g=GROUP),
                        in0=p_sb[:].rearrange("p (h b g) -> p h b g",
                                              h=H, b=BS, g=GROUP),
                        in1=mcols.unsqueeze(1).unsqueeze(3).broadcast_to(
                            [128, H, BS, GROUP]),
                        op=MULT)
                else:
                    for h in range(H):
                        nc.vector.tensor_tensor(
                            out=pm[:, h * 64:(h + 1) * 64].rearrange(
                                "p (b g) -> p b g", g=GROUP),
                            in0=p_sb[:, h * 64:(h + 1) * 64].rearrange(
                                "p (b g) -> p b g", g=GROUP),
                            in1=mcols.unsqueeze(2).broadcast_to(
                                [128, BS, GROUP]),
                            op=MULT)

                return pm

            onum = outp.tile([64, H, 129], f32)

            def group_pv(pm, vv, last):
                # num[bg, h, d] += sum_slot pm[slot, bg] [V | 1][slot, d]
                for h in range(H):
                    nc.tensor.matmul(
                        out=pv_out(h),
                        lhsT=pm[:, h * 64:(h + 1) * 64],
                        rhs=vv[:, h, :],
                        start=False, stop=False, skip_group_check=True)
                    if last:
                        if h % 2 == 0:
                            nc.vector.tensor_copy(onum[:, h, :], pv_out(h))
                        else:
                            nc.scalar.copy(onum[:, h, :], pv_out(h))
                        if h == 5:
                            nc.sync.dma_start(
                                num_d[:, 0:6 * 129],
                                onum[:, 0:6, :].rearrange(
                                    "p h d -> p (h d)"))
                        elif h == 7:
                            nc.sync.dma_start(
                                num_d[:, 6 * 129:],
                                onum[:, 6:8, :].rearrange(
                                    "p h d -> p (h d)"))

            def new_token_group():
                """16-partition group for the scattered new-token k/v rows:
                host-pretransposed K, ln-weight mask folded into the score
                PSUM via a one-hot matmul, no transposes."""
                scoresN = ps_s.tile([BS, QCOLS], f32, tag="scores")
                nc.tensor.matmul(out=scoresN[:], lhsT=logmN_sb[:],
                                 rhs=ind_sb[:],
                                 start=True, stop=False,
                                 skip_group_check=True)
                for h in range(H):
                    nc.tensor.matmul(
                        out=scoresN[:, h * 64:(h + 1) * 64],
                        lhsT=knewT[:, h * BS:(h + 1) * BS],
                        rhs=qt_sb[:, h * 64:(h + 1) * 64],
                        start=False, stop=(h == H - 1),
                        skip_group_check=True)
                pmN = ppool.tile([BS, QCOLS], bf16, tag="pm")
                nc.scalar.activation(pmN[:], scoresN[:], EXP, scale=SCALE)
                for h in range(H):
                    nc.tensor.matmul(
                        out=pv_out(h),
                        lhsT=pmN[:, h * 64:(h + 1) * 64],
                        rhs=vnew16[:, h, :],
                        start=False, stop=False, skip_group_check=True)

            # software pipeline: V conversions run VLEAD groups ahead of
            # compute (emitted after each group's ops so exp/mask never queue
            # behind a conversion waiting on DMA); K needs no conversion
            VLEAD = 2
            chunk_iter = iter(chunks)
            avail = []          # (kbuf, vbuf, j) per group
            ready_v = {}        # t -> vv

            first_pump = [True]

            def pump():
                off, r = next(chunk_iter)
                kbuf = kpool.tile([128, RMAX, D], f32r, tag="k")
                vbuf = vpool.tile([128, RMAX, D], f32r, tag="v")
                ksrc = kc[off:off + 128 * r, :].rearrange(
                    "(p j) d -> p j d", j=r)
                vsrc = vc[off:off + 128 * r, :].rearrange(
                    "(p j) d -> p j d", j=r)
                nc.sync.dma_start(
                    kbuf[:, 0:r, :].rearrange("p j d -> p (j d)"),
                    ksrc.rearrange("p j d -> p (j d)"))
                if first_pump[0]:
                    nc.sync.dma_start(ident[:], ident_d)
                    nc.sync.dma_start(qt_sb[:], qT_d)
                nc.sync.dma_start(
                    vbuf[:, 0:r, :].rearrange("p j d -> p (j d)"),
                    vsrc.rearrange("p j d -> p (j d)"))
                if first_pump[0]:
                    first_pump[0] = False
                    nc.gpsimd.dma_start(mask_sb[:], mask_d)
                    nc.gpsimd.dma_start(knewT[:], knewT_d)
                    nc.gpsimd.dma_start(
                        vnew16[:].rearrange("p h d -> p (h d)"), vnew_d)
                    nc.gpsimd.dma_start(logmN_sb[:], logmN_d)
                    nc.gpsimd.dma_start(ind_sb[:], ind_d)
                avail.extend((kbuf, vbuf, j) for j in range(r))
                return kbuf, vbuf

            def ensure(upto_v):
                while n_v[0] <= min(upto_v, NT - 1):
                    tvv = n_v[0]
                    while len(avail) <= tvv:
                        pump()
                    ready_v[tvv] = conv_v(avail[tvv][1],
                                          avail[tvv][2], tvv)
                    n_v[0] += 1

            n_v = [0]
            pms = {}
            kts = {}
            ensure(VLEAD - 1)

            def ksrc_of(t):
                while len(avail) <= t:
                    pump()
                kb, _, jj = avail[t]
                return kb[:, jj, :]

            kts[0] = transposes(ksrc_of(0))
            for t in range(NT):
                pms[t] = qk_front(kts.pop(t), t)
                if t + 1 < NT:
                    kts[t + 1] = transposes(ksrc_of(t + 1))
                if t > 0:
                    group_pv(pms.pop(t - 1), ready_v.pop(t - 1), False)
                if t == 2:
                    # constants have landed by now; slot the new-token
                    # group in without stalling the tensor engine
                    new_token_group()
                ensure(t + VLEAD)
            group_pv(pms.pop(NT - 1), ready_v.pop(NT - 1), True)

    nc.compile()
    return nc


def shard_inputs(q, k, v, k_cache, v_cache, slot_mapping, page_indices):
    """Host-side sharding: contiguous zero-copy cache slices per core plus
    small index-derived tensors (masks, transposed queries/new tokens)."""
    q = np.ascontiguousarray(np.asarray(q, dtype=np.float32))
    k = np.ascontiguousarray(np.asarray(k, dtype=np.float32))
    v = np.ascontiguousarray(np.asarray(v, dtype=np.float32))
    k_cache = np.asarray(k_cache, dtype=np.float32)
    v_cache = np.asarray(v_cache, dtype=np.float32)
    slot_mapping = np.asarray(slot_mapping, dtype=np.int64).ravel()
    page_indices = np.asarray(page_indices, dtype=np.int64)

    # qT[d, (h, b, g)] = q[b, h*GROUP+g, d]
    qr = q.reshape(BS, NUM_KV_HEADS, GROUP, HEAD_DIM)
    qT = np.ascontiguousarray(
        qr.transpose(3, 1, 0, 2).reshape(HEAD_DIM, QCOLS)
    ).astype(ml_dtypes.bfloat16)

    # ownership weights: count[slot, b] = multiplicity of slot in seq b's pages
    count = np.zeros((NUM_SLOTS, BS), dtype=np.float32)
    np.add.at(count,
              (page_indices.ravel(),
               np.repeat(np.arange(BS), KV_LEN)),
              1.0)
    # new-token slots: reference scatters k/v rows there BEFORE the gather, so
    # the streamed (old) row must contribute nothing; the new-token group
    # re-injects each referencing (seq, count) exactly once fleet-wide.
    # With duplicate slot_mapping entries the last writer wins (jax .at[].set).
    final_writer = {}
    for j in range(BS):
        final_writer[int(slot_mapping[j])] = j
    newcnt = np.zeros((BS, BS), dtype=np.float32)   # [new-token row j, seq b]
    for s, j in final_writer.items():
        newcnt[j, :] = count[s, :]
        count[s, :] = 0.0

    # new-token ln-weights (pre-divided by SCALE so
    # exp(SCALE*(s+w)) = exp(SCALE*s)*cnt); streamed groups use plain counts
    lognew = np.where(newcnt > 0,
                      np.log(np.maximum(newcnt, 1e-30)) / SCALE,
                      LOGBIG).astype(np.float32)            # [j, b]

    # new-token tiles, host-converted to the device layouts
    knewT = np.ascontiguousarray(
        k.reshape(BS, H, HEAD_DIM).transpose(2, 1, 0).reshape(HEAD_DIM, H * BS)
    ).astype(ml_dtypes.bfloat16)                            # [d, (h, j)]
    vnew16 = np.zeros((BS, H, 129), dtype=np.float32)
    vnew16[:, :, 0:128] = v.reshape(BS, H, HEAD_DIM)
    vnew16[:, :, 128] = 1.0
    vnew16 = np.ascontiguousarray(
        vnew16.reshape(BS, H * 129)).astype(ml_dtypes.bfloat16)

    # one-hot seq indicator [b', (h, b, g)]
    ind = np.zeros((BS, H, BS, GROUP), dtype=np.float32)
    ind[np.arange(BS), :, np.arange(BS), :] = 1.0
    ind = np.ascontiguousarray(ind.reshape(BS, QCOLS)).astype(
        ml_dtypes.bfloat16)

    # streamed slot (group t from chunk (off, r), partition p, sub j)
    #   = off + p*r + j ; group index t advances j-major within a chunk
    perm = np.empty(SLOTS_PER_CORE, dtype=np.int64)
    gi = 0
    off = 0
    for r in RS:
        idx = off + np.arange(128)[:, None] * r + np.arange(r)[None, :]
        for j in range(r):
            perm[gi * 128:(gi + 1) * 128] = idx[:, j]
            gi += 1
        off += 128 * r

    in_maps = []
    for c in range(N_CORES):
        base = c * SLOTS_PER_CORE
        mcore = count[base:base + SLOTS_PER_CORE][perm]     # [4096, 16]
        m = (mcore.reshape(NT, 128, BS).transpose(1, 0, 2)
             .reshape(128, NT * BS))
        # new-token rows: row j handled by core j % N_CORES
        lm_new = np.full((BS, BS), LOGBIG, dtype=np.float32)
        for j in range(BS):
            if j % N_CORES == c:
                lm_new[:, j] = lognew[j]
        in_maps.append({
            "k_shard": k_cache[base:base + SLOTS_PER_CORE],
            "v_shard": v_cache[base:base + SLOTS_PER_CORE],
            "k_newT": knewT,
            "v_new16": vnew16,
            "qT": qT,
            "mask": np.ascontiguousarray(m).astype(ml_dtypes.bfloat16),
            "logmN": lm_new.astype(ml_dtypes.bfloat16),
            "ind": ind,
            "ident": np.eye(128, dtype=np.float32),
        })
    return in_maps


_PROGS = {}
last_results = None  # BassKernelResults of the most recent kernel() call


def kernel(q, k, v, k_cache, v_cache, slot_mapping, page_indices):
    global last_results
    from concourse.bass_utils import run_bass_kernel_spmd

    in_maps = shard_inputs(q, k, v, k_cache, v_cache, slot_mapping,
                           page_indices)
    if "prog" not in _PROGS:
        try:
            _PROGS["prog"] = build_program(mask_4d=True)
        except Exception:
            _PROGS["prog"] = build_program(mask_4d=False)
    res = run_bass_kernel_spmd(_PROGS["prog"], in_maps,
                               core_ids=list(range(N_CORES)))
    last_results = res

    acc = np.zeros((64, H, 129), dtype=np.float64)
    for c in range(N_CORES):
        acc += res.results[c]["num"].astype(np.float64).reshape(64, H, 129)
    num = acc[:, :, 0:128]                      # [(b,g), h]
    den = acc[:, :, 128]                        # [(b,g), h]
    o = num / den[:, :, None]
    o = o.reshape(BS, GROUP, NUM_KV_HEADS, HEAD_DIM)   # [b, g, h, d]
    out = o.transpose(0, 2, 1, 3).reshape(BS, NUM_HEADS * HEAD_DIM)
    return np.ascontiguousarray(out.astype(np.float32))
